# revision 2
# baseline (speedup 1.0000x reference)
"""GQA causal attention with sinks (DeepseekV4Attention) on 8 TRN2 NeuronCores.

Problem: B=1, H=32, HKV=4, S=2048, D=128, fp32, causal + per-head sink logit.

Sharding (tensor-parallel on heads): core c owns query heads [4c, 4c+4) and
kv head c//2 (each kv head's group of 8 query heads spans exactly 2 cores).
attention_mask is causal; it is reproduced exactly on-device via affine_select
(masked probs underflow to 0.0 exactly, matching the -1e9 additive mask).

Per-core algorithm (4 heads, S=2048, D=128), scores kept TRANSPOSED
(k on partitions, q on free dim) so softmax-denominator reduction and PV both
run as full-rate f32r matmuls:
  scoresT[k,q] = KT.T @ QT      (KT,QT built by PE transposes, f32r)
  expT = exp(scale*scoresT)     (one ACT op per 2-chunk PSUM group)
  causal zeroing of diagonal chunks via gpsimd affine_select
  outT[d,q]  += V_kc.T @ expT   (V natural layout, f32r, PSUM-accumulated)
  sums[p,q]  += basis_p.T @ expT (per-panel row of a [4,512] PSUM tensor)
  out[q,d] = transpose(outT) * (1/(sums+exp(sink)))   then DMA to HBM.
"""
import sys
sys.path.insert(0, '/opt/trn_rl_repo')
from contextlib import ExitStack

import numpy as np

from concourse import bacc, bass, masks, mybir
from concourse.bass_utils import run_bass_kernel_spmd
from concourse.tile import TileContext

F32 = mybir.dt.float32
F32R = mybir.dt.float32r
EXPF = mybir.ActivationFunctionType.Exp

B, H, HKV, S, D = 1, 32, 4, 2048, 128
NCORES = 8
HL = H // NCORES          # 4 query heads per core
NP = S // 512             # 4 q-panels of 512 per head
NKC = S // 128            # 16 k-chunks of 128
SCALE = 1.0 / float(np.sqrt(D))

_nc_cache = None


def _build():
    nc = bacc.Bacc()
    q_in = nc.declare_dram_parameter("q", [HL * S, D], F32, isOutput=False)
    k_in = nc.declare_dram_parameter("k", [S, D], F32, isOutput=False)
    v_in = nc.declare_dram_parameter("v", [S, D], F32, isOutput=False)
    s_in = nc.declare_dram_parameter("sinks", [1, HL], F32, isOutput=False)
    o_out = nc.declare_dram_parameter("o", [S, HL * D], F32, isOutput=True)

    with TileContext(nc) as tc, ExitStack() as ctx:
        const = ctx.enter_context(tc.tile_pool(name="const", bufs=1))
        stage = ctx.enter_context(tc.tile_pool(name="stage", bufs=4))
        qtp = ctx.enter_context(tc.tile_pool(name="qtp", bufs=2))
        expp = ctx.enter_context(tc.tile_pool(name="expp", bufs=3))
        outp = ctx.enter_context(tc.tile_pool(name="outp", bufs=2))
        små = ctx.enter_context(tc.tile_pool(name="sml", bufs=2))
        ps_sc = ctx.enter_context(tc.tile_pool(name="ps_sc", bufs=2, space="PSUM"))
        ps_o = ctx.enter_context(tc.tile_pool(name="ps_o", bufs=1, space="PSUM"))
        ps_s = ctx.enter_context(tc.tile_pool(name="ps_s", bufs=1, space="PSUM"))
        ps_tr = ctx.enter_context(tc.tile_pool(name="ps_tr", bufs=2, space="PSUM"))

        ident = const.tile([128, 128], F32)
        masks.make_identity(nc, ident[:])

        # basis_p: [128,4] f32r, column p = 1.0 (softmax-sum stationaries)
        basis = []
        for p in range(NP):
            bf = const.tile([128, 4], F32, tag=f"basf{p}")
            nc.vector.memset(bf[:], 0.0)
            nc.vector.memset(bf[:, p:p + 1], 1.0)
            br = const.tile([128, 4], F32R, tag=f"basr{p}")
            nc.vector.tensor_copy(br[:], bf[:])
            basis.append(br)

        # exp(sinks) row [1, HL]
        snk = const.tile([1, HL], F32)
        nc.sync.dma_start(out=snk[:], in_=s_in[:])
        esnk = const.tile([1, HL], F32)
        nc.scalar.activation(esnk[:], snk[:], EXPF)

        # K^T and V (shared by all 4 heads of this core), f32r
        kt_sb = const.tile([128, S], F32R, tag="kt")
        v_sb = const.tile([128, S], F32R, tag="v")
        for kc in range(NKC):
            knat = stage.tile([128, 128], F32, tag="nat")
            nc.sync.dma_start(out=knat[:], in_=k_in[kc * 128:(kc + 1) * 128, :])
            ktp = ps_tr.tile([128, 128], F32, tag="tr")
            nc.tensor.transpose(ktp[:], knat[:], ident[:])
            nc.vector.tensor_copy(kt_sb[:, kc * 128:(kc + 1) * 128], ktp[:])
            vnat = stage.tile([128, 128], F32, tag="nat")
            nc.sync.dma_start(out=vnat[:], in_=v_in[kc * 128:(kc + 1) * 128, :])
            nc.scalar.copy(v_sb[:, kc * 128:(kc + 1) * 128], vnat[:])

        for h in range(HL):
            # Q^T for this head, f32r [128 D, S q]
            qt_sb = qtp.tile([128, S], F32R, tag="qt")
            for qt in range(NKC):
                qnat = stage.tile([128, 128], F32, tag="nat")
                nc.sync.dma_start(
                    out=qnat[:],
                    in_=q_in[h * S + qt * 128: h * S + (qt + 1) * 128, :])
                qp = ps_tr.tile([128, 128], F32, tag="tr")
                nc.tensor.transpose(qp[:], qnat[:], ident[:])
                nc.vector.tensor_copy(qt_sb[:, qt * 128:(qt + 1) * 128], qp[:])

            outt_head = outp.tile([128, S], F32, tag="outt")
            stacked = ps_s.tile([4, 512], F32)

            for p in range(NP):
                nkc = 4 * (p + 1)          # k-chunks in causal range
                outt_ps = ps_o.tile([128, 512], F32)
                for g in range(nkc // 2):
                    grp = ps_sc.tile([128, 1024], F32)
                    for i in range(2):
                        kc = 2 * g + i
                        nc.tensor.matmul(
                            out=grp[:, i * 512:(i + 1) * 512],
                            lhsT=kt_sb[:, kc * 128:(kc + 1) * 128],
                            rhs=qt_sb[:, p * 512:(p + 1) * 512],
                            start=True, stop=True)
                    egrp = expp.tile([128, 1024], F32R, tag="egrp")
                    nc.scalar.activation(egrp[:], grp[:], EXPF, scale=SCALE)
                    for i in range(2):
                        kc = 2 * g + i
                        esl = egrp[:, i * 512:(i + 1) * 512]
                        if kc >= 4 * p:  # diagonal chunk: causal zeroing
                            nc.gpsimd.affine_select(
                                out=esl, in_=esl,
                                compare_op=mybir.AluOpType.is_ge,
                                fill=0.0, base=512 * p - 128 * kc,
                                pattern=[[1, 512]], channel_multiplier=-1)
                        nc.tensor.matmul(
                            out=outt_ps[:], lhsT=v_sb[:, kc * 128:(kc + 1) * 128],
                            rhs=esl, start=(kc == 0), stop=(kc == nkc - 1),
                            skip_group_check=True)
                        nc.tensor.matmul(
                            out=stacked[:], lhsT=basis[p][:], rhs=esl,
                            start=(p == 0 and kc == 0),
                            stop=(p == NP - 1 and kc == nkc - 1),
                            skip_group_check=True)
                nc.vector.tensor_copy(outt_head[:, p * 512:(p + 1) * 512], outt_ps[:])

            # denominators: + exp(sink), transpose [4,512]->columns, reciprocal
            snk4 = små.tile([4, 1], F32, tag="snk4")
            nc.gpsimd.partition_broadcast(snk4[:], esnk[0:1, h:h + 1])
            stk_sb = små.tile([4, 512], F32, tag="stk")
            nc.vector.tensor_scalar_add(stk_sb[:], stacked[:], snk4[:])
            recip = små.tile([128, 16], F32, tag="recip")
            for t in range(4):
                trp = ps_tr.tile([128, 128], F32, tag="tr")
                nc.tensor.transpose(
                    trp[0:128, 0:4], stk_sb[0:4, t * 128:(t + 1) * 128],
                    ident[0:4, 0:4])
                nc.vector.reciprocal(recip[:, t * 4:(t + 1) * 4], trp[0:128, 0:4])

            # finalize: transpose outT back to [q,d], scale by recip, store
            for gq in range(NKC):
                pp, t = gq // 4, gq % 4
                top = ps_tr.tile([128, 128], F32, tag="tr")
                nc.tensor.transpose(
                    top[:], outt_head[:, gq * 128:(gq + 1) * 128], ident[:])
                oev = små.tile([128, 128], F32, tag="oev")
                c = 4 * t + pp
                nc.vector.tensor_scalar_mul(oev[:], top[:], recip[:, c:c + 1])
                nc.sync.dma_start(
                    out=o_out[gq * 128:(gq + 1) * 128, h * D:(h + 1) * D],
                    in_=oev[:])

    nc.finalize()
    return nc


def _get_nc():
    global _nc_cache
    if _nc_cache is None:
        _nc_cache = _build()
    return _nc_cache


def make_in_maps(query, key, value, sinks):
    q = np.asarray(query, dtype=np.float32).reshape(H, S, D)
    k = np.asarray(key, dtype=np.float32).reshape(HKV, S, D)
    v = np.asarray(value, dtype=np.float32).reshape(HKV, S, D)
    sk = np.asarray(sinks, dtype=np.float32).reshape(H)
    in_maps = []
    for c in range(NCORES):
        in_maps.append({
            "q": np.ascontiguousarray(q[HL * c:HL * (c + 1)]).reshape(HL * S, D),
            "k": np.ascontiguousarray(k[c // 2]),
            "v": np.ascontiguousarray(v[c // 2]),
            "sinks": np.ascontiguousarray(sk[HL * c:HL * (c + 1)]).reshape(1, HL),
        })
    return in_maps


def gather(results):
    out = np.empty((B, S, H, D), dtype=np.float32)
    for c in range(NCORES):
        out[0, :, HL * c:HL * (c + 1), :] = results[c]["o"].reshape(S, HL, D)
    return out


def kernel(query, key, value, attention_mask, sinks):
    nc = _get_nc()
    in_maps = make_in_maps(query, key, value, sinks)
    res = run_bass_kernel_spmd(nc, in_maps, list(range(NCORES))).results
    return gather(res)


# revision 8
# speedup vs baseline: 1.0265x; 1.0265x over previous
"""GQA causal attention with sinks (DeepseekV4Attention) on 8 TRN2 NeuronCores.

Problem: B=1, H=32, HKV=4, S=2048, D=128, fp32, causal + per-head sink logit.

Sharding (tensor-parallel on heads): core c owns query heads [4c, 4c+4) and
kv head c//2 (each kv head's group of 8 query heads spans exactly 2 cores).
attention_mask is causal; it is reproduced exactly on-device via affine_select
(masked probs underflow to 0.0 exactly, matching the -1e9 additive mask).

Per-core algorithm (4 heads, S=2048, D=128), scores kept TRANSPOSED
(k on partitions, q on free dim) so softmax-denominator reduction and PV both
run as full-rate f32r matmuls:
  scoresT[k,q] = KT.T @ QT      (KT,QT built by PE transposes, f32r)
  expT = exp(scale*scoresT)     (one ACT op per 2-chunk PSUM group)
  causal zeroing of diagonal chunks via gpsimd affine_select
  outT[d,q]  += V_kc.T @ expT   (V natural layout, f32r, PSUM-accumulated)
  denominators: per chunk either a basis-matmul on PE into a [4,512] PSUM
  (row = panel) or a DVE elementwise accumulate (PE/DVE load balance knob),
  DVE accumulators folded in by one basis-matmul per panel.
  out[q,d] = transpose(outT) * (1/(sums+exp(sink)))   then DMA to HBM.

Engines execute their instruction streams in order, so the emission order IS
the software pipeline: each steady-state group emits exp(g), QK(g+1), then
PV/sum(g), and one next-head QT-build step plus one previous-head output
finalization step are sprinkled into every group so head boundaries don't
serialize. All HBM traffic is batched: one DMA per K/V/Q-head/out-head.
"""
import sys
sys.path.insert(0, '/opt/trn_rl_repo')
from contextlib import ExitStack

import numpy as np

from concourse import bacc, bass, masks, mybir
from concourse.bass_utils import run_bass_kernel_spmd
from concourse.tile import TileContext

F32 = mybir.dt.float32
F32R = mybir.dt.float32r
EXPF = mybir.ActivationFunctionType.Exp

B, H, HKV, S, D = 1, 32, 4, 2048, 128
NCORES = 8
HL = H // NCORES          # 4 query heads per core
NP = S // 512             # 4 q-panels of 512 per head
NKC = S // 128            # 16 k-chunks of 128
SCALE = 1.0 / float(np.sqrt(D))
# denominator-reduction load balance: fraction of chunks handled by each
# engine (PE basis-matmul / DVE accumulate / GPSIMD accumulate)
SUM_FRAC_DVE = 0.40
SUM_FRAC_GPS = 0.60
V_COPY_ENGINE = "vector"  # "vector" (DVE) or "scalar" (ACT)

_nc_cache = None


def _build():
    nc = bacc.Bacc()
    q_in = nc.declare_dram_parameter("q", [HL * S, D], F32, isOutput=False)
    k_in = nc.declare_dram_parameter("k", [S, D], F32, isOutput=False)
    v_in = nc.declare_dram_parameter("v", [S, D], F32, isOutput=False)
    s_in = nc.declare_dram_parameter("sinks", [1, HL], F32, isOutput=False)
    o_out = nc.declare_dram_parameter("o", [S, HL * D], F32, isOutput=True)

    with TileContext(nc) as tc, ExitStack() as ctx:
        const = ctx.enter_context(tc.tile_pool(name="const", bufs=1))
        qstgp = ctx.enter_context(tc.tile_pool(name="qstgp", bufs=2))
        qtp = ctx.enter_context(tc.tile_pool(name="qtp", bufs=2))
        expp = ctx.enter_context(tc.tile_pool(name="expp", bufs=3))
        outp = ctx.enter_context(tc.tile_pool(name="outp", bufs=2))
        accp = ctx.enter_context(tc.tile_pool(name="accp", bufs=2))
        sml = ctx.enter_context(tc.tile_pool(name="sml", bufs=2))
        ps_sc = ctx.enter_context(tc.tile_pool(name="ps_sc", bufs=2, space="PSUM"))
        ps_o = ctx.enter_context(tc.tile_pool(name="ps_o", bufs=1, space="PSUM"))
        ps_s = ctx.enter_context(tc.tile_pool(name="ps_s", bufs=1, space="PSUM"))
        ps_tr = ctx.enter_context(tc.tile_pool(name="ps_tr", bufs=2, space="PSUM"))

        ident = const.tile([128, 128], F32)
        masks.make_identity(nc, ident[:])

        # basis_p: [128,4] f32r, column p = 1.0 (softmax-sum stationaries)
        basis = []
        for p in range(NP):
            bf = const.tile([128, 4], F32, tag=f"basf{p}")
            nc.vector.memset(bf[:], 0.0)
            nc.vector.memset(bf[:, p:p + 1], 1.0)
            br = const.tile([128, 4], F32R, tag=f"basr{p}")
            nc.vector.tensor_copy(br[:], bf[:])
            basis.append(br)

        # exp(sinks) row [1, HL]
        snk = const.tile([1, HL], F32)
        nc.sync.dma_start(out=snk[:], in_=s_in[:])
        esnk = const.tile([1, HL], F32)
        nc.scalar.activation(esnk[:], snk[:], EXPF)

        # K and V staged via one batched DMA each: [128 row, chunk, col]
        knat = const.tile([128, S], F32, tag="knat")
        vnat = const.tile([128, S], F32, tag="vnat")
        nc.sync.dma_start(
            out=knat[:].rearrange("p (c d) -> p c d", d=128),
            in_=k_in[:].rearrange("(c p) d -> p c d", p=128))
        nc.sync.dma_start(
            out=vnat[:].rearrange("p (c d) -> p c d", d=128),
            in_=v_in[:].rearrange("(c p) d -> p c d", p=128))

        kt_sb = const.tile([128, S], F32R, tag="kt")
        v_sb = const.tile([128, S], F32R, tag="v")
        for kc in range(NKC):
            sl = slice(kc * 128, (kc + 1) * 128)
            ktp = ps_tr.tile([128, 128], F32, tag="tr")
            nc.tensor.transpose(ktp[:], knat[:, sl], ident[:])
            nc.vector.tensor_copy(kt_sb[:, sl], ktp[:])
            if V_COPY_ENGINE == "scalar":
                nc.scalar.copy(v_sb[:, sl], vnat[:, sl])
            else:
                nc.vector.tensor_copy(v_sb[:, sl], vnat[:, sl])

        # ---- per-head state handed between pipeline phases ----
        qstg_tiles = [None] * HL    # staged natural-layout Q per head
        qt_tiles = [None] * HL      # f32r [128, S] Q^T per head
        fin_state = {}              # head -> (outt_head, recip, ostg)

        def emit_q_dma(h):
            qstg_tiles[h] = qstgp.tile([128, S], F32, tag="qstg", name=f"qs{h}")
            nc.sync.dma_start(
                out=qstg_tiles[h][:].rearrange("p (c d) -> p c d", d=128),
                in_=q_in[h * S:(h + 1) * S, :].rearrange("(c p) d -> p c d", p=128))

        def emit_qt_step(h, qt):
            """One step of building head h's Q^T (PE transpose -> evac)."""
            if qt == 0:
                qt_tiles[h] = qtp.tile([128, S], F32R, tag="qt", name=f"qt{h}")
            qp = ps_tr.tile([128, 128], F32, tag="tr")
            nc.tensor.transpose(
                qp[:], qstg_tiles[h][:, qt * 128:(qt + 1) * 128], ident[:])
            nc.vector.tensor_copy(qt_tiles[h][:, qt * 128:(qt + 1) * 128], qp[:])

        def emit_fin_step(h, gq):
            """One step of finalizing head h's output: transpose outT back to
            [q,d], scale by 1/denominator into the per-head out staging."""
            outt_head, recip, ostg = fin_state[h]
            pp, t = gq // 4, gq % 4
            top = ps_tr.tile([128, 128], F32, tag="tr")
            nc.tensor.transpose(
                top[:], outt_head[:, gq * 128:(gq + 1) * 128], ident[:])
            c = 4 * t + pp
            nc.vector.tensor_scalar_mul(
                ostg[:, gq * 128:(gq + 1) * 128], top[:], recip[:, c:c + 1])
            if gq == NKC - 1:  # one batched store for the whole head
                nc.sync.dma_start(
                    out=o_out[:, h * D:(h + 1) * D].rearrange(
                        "(c p) d -> p c d", p=128),
                    in_=ostg[:].rearrange("p (c d) -> p c d", d=128))

        # head 0's Q staged+transposed upfront (overlaps the K/V setup above)
        emit_q_dma(0)
        if HL > 1:
            emit_q_dma(1)
        for qt in range(NKC):
            emit_qt_step(0, qt)

        dve_pick = 0.0
        gps_pick = 0.0
        for h in range(HL):
            qt_sb = qt_tiles[h]
            outt_head = outp.tile([128, S], F32, tag="outt")
            stacked = ps_s.tile([4, 512], F32)
            if h + 2 < HL:
                emit_q_dma(h + 2)

            seq = [(p, g) for p in range(NP) for g in range(2 * (p + 1))]
            started = [False]

            def emit_qk(idx):
                p, g = seq[idx]
                grp = ps_sc.tile([128, 1024], F32, tag="grp")
                for i in range(2):
                    kc = 2 * g + i
                    nc.tensor.matmul(
                        out=grp[:, i * 512:(i + 1) * 512],
                        lhsT=kt_sb[:, kc * 128:(kc + 1) * 128],
                        rhs=qt_sb[:, p * 512:(p + 1) * 512],
                        start=True, stop=True)
                return grp

            grp = emit_qk(0)
            acc_dve = acc_gps = None
            pend_gps = []
            for idx, (p, g) in enumerate(seq):
                nkc = 4 * (p + 1)
                last_of_panel = (g == 2 * (p + 1) - 1)
                if g == 0:
                    outt_ps = ps_o.tile([128, 512], F32)
                    acc_dve = acc_gps = None
                egrp = expp.tile([128, 1024], F32R, tag="egrp")
                nc.scalar.activation(egrp[:], grp[:], EXPF, scale=SCALE)
                # causal zeroing first so Pool doesn't convoy PV behind adds
                for i in range(2):
                    kc = 2 * g + i
                    if kc >= 4 * p:
                        esl = egrp[:, i * 512:(i + 1) * 512]
                        nc.gpsimd.affine_select(
                            out=esl, in_=esl,
                            compare_op=mybir.AluOpType.is_ge,
                            fill=0.0, base=512 * p - 128 * kc,
                            pattern=[[1, 512]], channel_multiplier=-1)
                if idx + 1 < len(seq):
                    grp = emit_qk(idx + 1)     # lookahead: PE fills ACT latency
                # sprinkled PE work here also absorbs the exp->PV latency
                if h + 1 < HL and idx < NKC:
                    emit_qt_step(h + 1, idx)
                if h - 1 in fin_state and idx < NKC:
                    emit_fin_step(h - 1, idx)
                    if idx == NKC - 1:
                        del fin_state[h - 1]
                # gpsimd sum-adds delayed one group (drained at panel end)
                for esl_pend in pend_gps:
                    if acc_gps is None:
                        acc_gps = accp.tile([128, 512], F32R, tag="accg",
                                            name=f"accg{h}_{p}")
                        nc.gpsimd.tensor_copy(acc_gps[:], esl_pend)
                    else:
                        nc.gpsimd.tensor_add(acc_gps[:], acc_gps[:], esl_pend)
                pend_gps = []
                for i in range(2):
                    kc = 2 * g + i
                    esl = egrp[:, i * 512:(i + 1) * 512]
                    nc.tensor.matmul(
                        out=outt_ps[:], lhsT=v_sb[:, kc * 128:(kc + 1) * 128],
                        rhs=esl, start=(kc == 0), stop=(kc == nkc - 1),
                        skip_group_check=True)
                    # denominator: DVE or GPSIMD accumulate (balance knob)
                    dve_pick += SUM_FRAC_DVE
                    if dve_pick >= 1.0:
                        dve_pick -= 1.0
                        if acc_dve is None:
                            acc_dve = accp.tile([128, 512], F32R, tag="accd",
                                                name=f"accd{h}_{p}")
                            nc.vector.tensor_copy(acc_dve[:], esl)
                        else:
                            nc.vector.tensor_add(acc_dve[:], acc_dve[:], esl)
                    else:
                        pend_gps.append(esl)
                if last_of_panel:
                    for esl_pend in pend_gps:
                        if acc_gps is None:
                            acc_gps = accp.tile([128, 512], F32R, tag="accg",
                                                name=f"accg{h}_{p}")
                            nc.gpsimd.tensor_copy(acc_gps[:], esl_pend)
                        else:
                            nc.gpsimd.tensor_add(acc_gps[:], acc_gps[:], esl_pend)
                    pend_gps = []
                if last_of_panel:
                    accs = [a for a in (acc_dve, acc_gps) if a is not None]
                    assert accs, "every panel must have an accumulator"
                    for ai, a in enumerate(accs):
                        nc.tensor.matmul(
                            out=stacked[:], lhsT=basis[p][:], rhs=a[:],
                            start=not started[0],
                            stop=(p == NP - 1 and ai == len(accs) - 1),
                            skip_group_check=True)
                        started[0] = True
                    if p == NP - 1 and not accs:
                        pass  # stop carried by a PE-path chunk below
                    nc.vector.tensor_copy(
                        outt_head[:, p * 512:(p + 1) * 512], outt_ps[:])

            # denominators: + exp(sink), transpose [4,512]->columns, reciprocal
            snk4 = sml.tile([4, 1], F32, tag="snk4")
            nc.gpsimd.partition_broadcast(snk4[:], esnk[0:1, h:h + 1])
            stk_sb = sml.tile([4, 512], F32, tag="stk")
            nc.vector.tensor_scalar_add(stk_sb[:], stacked[:], snk4[:])
            recip = sml.tile([128, 16], F32, tag="recip")
            for t in range(4):
                trp = ps_tr.tile([128, 128], F32, tag="tr")
                nc.tensor.transpose(
                    trp[0:128, 0:4], stk_sb[0:4, t * 128:(t + 1) * 128],
                    ident[0:4, 0:4])
                nc.vector.reciprocal(recip[:, t * 4:(t + 1) * 4], trp[0:128, 0:4])
            ostg = sml.tile([128, S], F32, tag="ostg", name=f"ostg{h}")
            fin_state[h] = (outt_head, recip, ostg)

        # drain remaining finalization (last head)
        for h in sorted(fin_state):
            for gq in range(NKC):
                emit_fin_step(h, gq)

    nc.finalize()
    return nc


def _get_nc():
    global _nc_cache
    if _nc_cache is None:
        _nc_cache = _build()
    return _nc_cache


def make_in_maps(query, key, value, sinks):
    q = np.asarray(query, dtype=np.float32).reshape(H, S, D)
    k = np.asarray(key, dtype=np.float32).reshape(HKV, S, D)
    v = np.asarray(value, dtype=np.float32).reshape(HKV, S, D)
    sk = np.asarray(sinks, dtype=np.float32).reshape(H)
    in_maps = []
    for c in range(NCORES):
        in_maps.append({
            "q": np.ascontiguousarray(q[HL * c:HL * (c + 1)]).reshape(HL * S, D),
            "k": np.ascontiguousarray(k[c // 2]),
            "v": np.ascontiguousarray(v[c // 2]),
            "sinks": np.ascontiguousarray(sk[HL * c:HL * (c + 1)]).reshape(1, HL),
        })
    return in_maps


def gather(results):
    out = np.empty((B, S, H, D), dtype=np.float32)
    for c in range(NCORES):
        out[0, :, HL * c:HL * (c + 1), :] = results[c]["o"].reshape(S, HL, D)
    return out


def kernel(query, key, value, attention_mask, sinks):
    nc = _get_nc()
    in_maps = make_in_maps(query, key, value, sinks)
    res = run_bass_kernel_spmd(nc, in_maps, list(range(NCORES))).results
    return gather(res)
